# revision 26
# baseline (speedup 1.0000x reference)
"""ChannelAffinityAttention Trainium2 kernel.

Full-input contract: kernel(**inputs) takes the unsharded numpy inputs
and returns the full [16, 256, 128, 128] output. Internally the batch
dim (16) is sharded across 8 NeuronCores (2 per core); the tiny MLP
params are replicated.

Per-core dataflow (all shapes hardcoded):
  text path (tiny, overlaps the visual stream):
    tfT = transpose(text_feat[b]) via PE             [768, 64] in 6 chunks
    hiddenT = relu(W1.T @ tfT + b1)                  [64, 64]
    projT_h = W2[:, half].T @ hiddenT + b2[half]     [128, 64] per c-half
  visual path, per unit (b, c-half) = [128 ch, 16384 px]:
    stream 4x [128, 4096] chunks HBM->SBUF, partial reduce_sum each
    vis_scale = total_sum / (HW * T); aff = projT * vis_scale + maskbias
    softmax over tokens (exp with accum_out); cw = sum(ex*projT)/sum(ex)
    g = sigmoid(cw); chunk *= g in place; stream back to HBM
"""

import numpy as np

import concourse.bass as bass
import concourse.tile as tile
from concourse.tile import add_dep_helper
from concourse import masks, mybir
from concourse.bass_utils import run_bass_kernel_spmd

F32 = mybir.dt.float32
I32 = mybir.dt.int32
AX = mybir.AxisListType
AF = mybir.ActivationFunctionType
ALU = mybir.AluOpType

N_CORES = 8
B, C, H, W = 16, 256, 128, 128
N, D, MID = 64, 768, 64
B_PER = B // N_CORES          # 2 batches per core
HW = H * W                    # 16384
P = 128                       # SBUF partitions / channel-block size
NHALF = C // P                # 2 channel halves
KD = D // P                   # 6 contraction chunks for W1
FCH = 4096                    # free-dim chunk of a visual unit
NCH = HW // FCH               # 4 chunks per unit
NEG_BIG = -1.0e30


def _split_multi_waits(nc, max_waits=1):
    # The walrus build here rejects instructions carrying >1 sync-wait
    # ("Too many sync wait commands"); hoist extras onto standalone
    # event-semaphore instructions right before the original.
    n = 0
    for fn in nc.m.functions:
        for blk in fn.blocks:
            out = []
            for inst in blk.instructions:
                si = inst.sync_info
                waits = list(si.on_wait) if si and si.on_wait else []
                if len(waits) > max_waits:
                    for wv in waits[:-max_waits]:
                        n += 1
                        out.append(
                            mybir.InstEventSemaphore(
                                name=f"waitfix-{n}-{inst.name}",
                                engine=inst.engine,
                                ins=[],
                                outs=[],
                                sync_info=mybir.SyncInfo(
                                    on_wait=[wv], on_update=[]
                                ),
                            )
                        )
                    inst.sync_info = mybir.SyncInfo(
                        on_wait=waits[-max_waits:],
                        on_update=list(si.on_update or []),
                    )
                out.append(inst)
            blk.instructions[:] = out
    return n



def _lean_drain_and_barrier(self, tick_clock, wait_clock):
    # Same as TileContext._drain_and_barrier but without the second
    # all-engine barrier: after the single barrier + sem clear, every
    # engine stream simply ends, and NRT won't start a re-execution
    # until all engines (incl. the one doing the clear) have halted.
    from concourse.vector_clock import ScopedClock

    drain_inst = self.nc.sync.drain()
    wait_clock.add_sem_waits(
        drain_inst.ins, ScopedClock({None: tick_clock.global_clock})
    )
    self.nc.all_engine_barrier()
    popped = self.nc._tile_sem_poison_stack.pop()
    assert popped is self._sem_poison
    self.nc.clear_and_free_semaphores(list(self.sems.allocated().values()))


def _build():
    nc = bass.Bass()
    vis = nc.declare_dram_parameter("visual", [B_PER, C, H, W], F32, isOutput=False)
    txt = nc.declare_dram_parameter("text", [B_PER, N, D], F32, isOutput=False)
    msk = nc.declare_dram_parameter("mask", [B_PER, N], I32, isOutput=False)
    w1p = nc.declare_dram_parameter("W1p", [P, KD * MID], F32, isOutput=False)
    b1 = nc.declare_dram_parameter("b1", [MID], F32, isOutput=False)
    w2 = nc.declare_dram_parameter("W2", [MID, C], F32, isOutput=False)
    b2 = nc.declare_dram_parameter("b2", [C], F32, isOutput=False)
    temp = nc.declare_dram_parameter("temperature", [1], F32, isOutput=False)
    out = nc.declare_dram_parameter("out", [B_PER, C, H, W], F32, isOutput=True)

    visf = vis.rearrange("b c h w -> b c (h w)")
    outf = out.rearrange("b c h w -> b c (h w)")

    orig_drain = tile.TileContext._drain_and_barrier
    tile.TileContext._drain_and_barrier = _lean_drain_and_barrier
    try:
        _build_body(nc, visf, outf, txt, msk, w1p, b1, w2, b2, temp)
    finally:
        tile.TileContext._drain_and_barrier = orig_drain
    _split_multi_waits(nc)
    return nc


def _build_body(nc, visf, outf, txt, msk, w1p, b1, w2, b2, temp):
    with tile.TileContext(nc) as tc:
        with (
            tc.tile_pool(name="big", bufs=11) as big,
            tc.tile_pool(name="small", bufs=1) as small,
            tc.tile_pool(name="txtp", bufs=2) as txtp,
            tc.tile_pool(name="punit", bufs=2) as punit,
            tc.tile_pool(name="ptp", bufs=2, space="PSUM") as ptp,
            tc.tile_pool(name="pacc", bufs=1, space="PSUM") as pacc,
            tc.tile_pool(name="pproj", bufs=2, space="PSUM") as pproj,
        ):
            # ---- replicated params ----
            # All small loads go on the scalar HWDGE ring (idle early; the
            # sync ring is reserved for the visual stream) as few, fat
            # descriptors. W1 is pre-packed on host to the [p, (k m)]
            # lhsT-chunk layout so it loads as one contiguous 2D DMA.
            w1s = small.tile([P, KD * MID], F32, tag="w1s")
            nc.gpsimd.dma_start(out=w1s, in_=w1p[:, :])
            # W2 with b2 appended as a 65th contraction row: the proj
            # matmul then adds b2 via a ones row in the rhs.
            w2e = small.tile([MID + 1, C], F32, tag="w2e")
            nc.gpsimd.dma_start(out=w2e[:MID, :], in_=w2[:, :])
            nc.gpsimd.dma_start(
                out=w2e[MID:MID + 1, :], in_=b2.rearrange("(o c) -> o c", o=1)
            )
            # b1 as a single-row lhsT: a 7th accumulation matmul
            # (b1row.T @ ones) adds it to every token column.
            b1r = small.tile([1, MID], F32, tag="b1r")
            nc.gpsimd.dma_start(out=b1r, in_=b1.rearrange("(o m) -> o m", o=1))
            ones1 = small.tile([1, N], F32, tag="ones1")
            nc.vector.memset(ones1, 1.0)
            onesP = small.tile([1, P], F32, tag="onesP")
            nc.vector.memset(onesP, 1.0)
            # rT = 1 / (HW * T) on one partition, then PE-broadcast to 128
            tq = small.tile([1, 1], F32, tag="tq")
            nc.gpsimd.dma_start(out=tq, in_=temp.rearrange("(o t) -> o t", o=1))
            r1 = small.tile([1, 1], F32, tag="r1")
            nc.vector.reciprocal(r1, tq)
            nc.vector.tensor_scalar_mul(r1, r1, 1.0 / float(HW))
            prt = pproj.tile([P, 1], F32, tag="pbc")
            nc.tensor.matmul(prt, lhsT=onesP, rhs=r1, start=True, stop=True)
            rT = small.tile([P, 1], F32, tag="rT")
            nc.scalar.copy(rT, prt)
            ident = small.tile([MID, MID], F32, tag="ident")
            masks.make_identity(nc, ident[:])

            # ---- visual pipeline setup + prologue ----
            # Emitted BEFORE the text path so the early load triggers sit
            # ahead of the text ACT ops in the scalar engine stream (the
            # scalar ring carries the odd-j prologue loads). Two HWDGE
            # rings: a single ring leaves per-transfer completion bubbles
            # (~355 GB/s); two rings sustain ~431 GB/s.
            units = [(b, h) for b in range(B_PER) for h in range(NHALF)]
            NU = len(units)
            vts = {u: [] for u in range(NU)}
            psums = {}
            rings = [nc.sync, nc.scalar]

            def load_chunk(u, j, ring):
                b, h = units[u]
                cs = h * P
                vt = big.tile([P, FCH], F32, tag="vt", name=f"vt{u}_{j}")
                ring.dma_start(
                    out=vt, in_=visf[b, cs:cs + P, j * FCH:(j + 1) * FCH]
                )
                vts[u].append(vt)

            def reduce_chunk(u, j):
                return nc.vector.reduce_sum(
                    psums[u][:, j:j + 1], vts[u][j], axis=AX.X
                )

            def load_reduce(u, j, ring=None):
                load_chunk(u, j, ring or nc.sync)
                return reduce_chunk(u, j)

            # text inputs load first on the scalar ring (tiny; they gate
            # the softmax chain of unit 0 and must not queue behind the
            # 8 MiB of prologue visual loads)
            tfs, mrows = [], []
            for b in range(B_PER):
                tf = txtp.tile([N, D], F32, tag="tf", name=f"tf{b}")
                nc.gpsimd.dma_start(out=tf, in_=txt[b])
                tfs.append(tf)
                mrow = txtp.tile([1, N], I32, tag="mrow", name=f"mrow{b}")
                nc.gpsimd.dma_start(
                    out=mrow, in_=msk[b].rearrange("(o n) -> o n", o=1)
                )
                mrows.append(mrow)

            # prologue: units 0 and 1 load on both rings; unit 0 reduces
            # now, unit 1's reduces are dep-pinned into phase 0
            psums[0] = punit.tile([P, NCH], F32, tag="psums", name="psums0")
            psums[1] = punit.tile([P, NCH], F32, tag="psums", name="psums1")
            for j in range(NCH):
                load_reduce(0, j, ring=rings[j % 2])
            for j in range(NCH):
                load_chunk(1, j, rings[j % 2])

            # ---- text path: projT[b][h] = (proj(text[b]) + b2).T ----
            projT = {}
            biasb = []
            for b in range(B_PER):
                tf = tfs[b]
                mrow = mrows[b]
                brow = txtp.tile([1, N], F32, tag="brow")
                nc.gpsimd.tensor_scalar(
                    out=brow, in0=mrow, scalar1=0, scalar2=NEG_BIG,
                    op0=ALU.is_equal, op1=ALU.mult,
                )
                pbb = pproj.tile([P, N], F32, tag="pbc")
                nc.tensor.matmul(pbb, lhsT=onesP, rhs=brow, start=True, stop=True)
                bb = small.tile([P, N], F32, tag=f"biasb{b}")
                nc.scalar.copy(bb, pbb)
                biasb.append(bb)

                tfTs = []
                for k in range(KD):
                    tps = ptp.tile([P, N], F32, tag="tps")
                    nc.tensor.transpose(tps, tf[:, k * P:(k + 1) * P], ident)
                    tfT = txtp.tile([P, N], F32, tag="tfT", bufs=3)
                    nc.scalar.copy(tfT, tps)
                    tfTs.append(tfT)
                hacc = pacc.tile([MID, N], F32, tag="hacc")
                for k in range(KD):
                    nc.tensor.matmul(
                        hacc,
                        lhsT=w1s[:, k * MID:(k + 1) * MID],
                        rhs=tfTs[k],
                        start=(k == 0),
                        stop=False,
                    )
                nc.tensor.matmul(hacc, lhsT=b1r, rhs=ones1, start=False, stop=True)
                # relu(hacc) into rows 0..MID of hTp; row MID is ones so the
                # proj matmul's 65th contraction row adds b2.
                hTp = txtp.tile([MID + 1, N], F32, tag="hTp")
                nc.scalar.activation(hTp[:MID, :], hacc, AF.Relu)
                nc.gpsimd.memset(hTp[MID:MID + 1, :], 1.0)
                for h in range(NHALF):
                    pp = pproj.tile([P, N], F32, tag="pp")
                    nc.tensor.matmul(
                        pp, lhsT=w2e[:, h * P:(h + 1) * P], rhs=hTp,
                        start=True, stop=True,
                    )
                    pt = small.tile([P, N], F32, tag=f"projT{b}{h}")
                    nc.scalar.copy(pt, pp)
                    projT[(b, h)] = pt

            for u in range(NU):
                b, h = units[u]
                cs = h * P
                # softmax chain -> per-channel gate g (ACT only uses Exp:
                # sigmoid is computed as 1/(1+exp(-x)) to avoid ACT
                # table reloads on the critical path)
                vsum = punit.tile([P, 1], F32, tag="vsum")
                nc.vector.reduce_sum(vsum, psums[u], axis=AX.X)
                vs = punit.tile([P, 1], F32, tag="vs")
                nc.vector.tensor_mul(vs, vsum, rT)
                aff = punit.tile([P, N], F32, tag="aff")
                nc.vector.tensor_scalar_mul(aff, projT[(b, h)], vs)
                nc.vector.tensor_add(aff, aff, biasb[b])
                nmx = punit.tile([P, 1], F32, tag="nmx")
                nc.vector.reduce_max(nmx, aff, axis=AX.X, negate=True)
                ex = punit.tile([P, N], F32, tag="ex")
                sx = punit.tile([P, 1], F32, tag="sx")
                nc.scalar.activation(
                    ex, aff, AF.Exp, bias=nmx, scale=1.0, accum_out=sx
                )
                xp = punit.tile([P, N], F32, tag="xp")
                swx = punit.tile([P, 1], F32, tag="swx")
                nc.vector.tensor_mul(xp, ex, projT[(b, h)])
                nc.vector.reduce_sum(swx, xp, axis=AX.X)
                rs = punit.tile([P, 1], F32, tag="rs")
                nc.vector.reciprocal(rs, sx)
                cw = punit.tile([P, 1], F32, tag="cw")
                nc.vector.tensor_mul(cw, swx, rs)
                en = punit.tile([P, 1], F32, tag="en")
                nc.scalar.activation(en, cw, AF.Exp, scale=-1.0)
                g = punit.tile([P, 1], F32, tag="g")
                nc.vector.tensor_scalar_add(g, en, 1.0)
                nc.vector.reciprocal(g, g)

                if u + 2 < NU:
                    psums[u + 2] = punit.tile(
                        [P, NCH], F32, tag="psums", name=f"psums{u + 2}"
                    )
                for j in range(NCH):
                    s_i = nc.vector.tensor_scalar_mul(vts[u][j], vts[u][j], g)
                    nc.scalar.dma_start(
                        out=outf[b, cs:cs + P, j * FCH:(j + 1) * FCH],
                        in_=vts[u][j],
                    )
                    # units u, u+1 are resident (prologue/previous phase);
                    # this phase streams in u+2 and reduces u+1
                    if u + 2 < NU:
                        load_chunk(u + 2, j, nc.sync)
                    if u + 1 < NU:
                        r_i = reduce_chunk(u + 1, j)
                        # pin DVE order: this unit's scale (and therefore the
                        # softmax chain before it) must precede the next
                        # unit's bulk reduce, or the scheduler slots the
                        # 4.4us reduce into the chain and delays the stores
                        add_dep_helper(r_i.ins, s_i.ins, sync=False,
                                       reason="chain+scale before next reduce")


_NC_CACHE = None


def make_in_maps(inputs):
    visual = np.asarray(inputs["visual_feat"], dtype=np.float32)
    text = np.asarray(inputs["text_feat"], dtype=np.float32)
    mask = np.asarray(inputs["text_mask"], dtype=np.int32)
    w1 = np.asarray(inputs["W1"], dtype=np.float32)
    # pack W1 [768, 64] -> [128, 6*64] so lhsT chunk k for the contraction
    # over D lives at w1p[:, k*64:(k+1)*64] and loads as one contiguous DMA
    w1p = np.ascontiguousarray(
        w1.reshape(KD, P, MID).transpose(1, 0, 2).reshape(P, KD * MID))
    b1 = np.asarray(inputs["b1"], dtype=np.float32)
    w2 = np.asarray(inputs["W2"], dtype=np.float32)
    b2 = np.asarray(inputs["b2"], dtype=np.float32)
    temp = np.asarray(inputs["temperature"], dtype=np.float32)

    in_maps = []
    for i in range(N_CORES):
        s = slice(i * B_PER, (i + 1) * B_PER)
        in_maps.append({
            "visual": np.ascontiguousarray(visual[s]),
            "text": np.ascontiguousarray(text[s]),
            "mask": np.ascontiguousarray(mask[s]),
            "W1p": w1p, "b1": b1, "W2": w2, "b2": b2, "temperature": temp,
        })
    return in_maps


def kernel(**inputs):
    global _NC_CACHE
    if _NC_CACHE is None:
        _NC_CACHE = _build()
    nc = _NC_CACHE
    in_maps = make_in_maps(inputs)
    res = run_bass_kernel_spmd(nc, in_maps, list(range(N_CORES)))
    return np.concatenate([res.results[i]["out"] for i in range(N_CORES)], axis=0)


# revision 27
# speedup vs baseline: 1.1933x; 1.1933x over previous
"""ChannelAffinityAttention Trainium2 kernel.

Full-input contract: kernel(**inputs) takes the unsharded numpy inputs
and returns the full [16, 256, 128, 128] output. Internally the batch
dim (16) is sharded across 8 NeuronCores (2 per core); the tiny MLP
params are replicated.

Per-core dataflow (all shapes hardcoded):
  text path (tiny, overlaps the visual stream):
    tfT = transpose(text_feat[b]) via PE             [768, 64] in 6 chunks
    hiddenT = relu(W1.T @ tfT + b1)                  [64, 64]
    projT_h = W2[:, half].T @ hiddenT + b2[half]     [128, 64] per c-half
  visual path, per unit (b, c-half) = [128 ch, 16384 px]:
    stream 4x [128, 4096] chunks HBM->SBUF, partial reduce_sum each
    vis_scale = total_sum / (HW * T); aff = projT * vis_scale + maskbias
    softmax over tokens (exp with accum_out); cw = sum(ex*projT)/sum(ex)
    g = sigmoid(cw); chunk *= g in place; stream back to HBM
"""

import numpy as np

import concourse.bass as bass
import concourse.tile as tile
from concourse.tile import add_dep_helper
from concourse import masks, mybir
from concourse.bass_utils import run_bass_kernel_spmd

F32 = mybir.dt.float32
I32 = mybir.dt.int32
AX = mybir.AxisListType
AF = mybir.ActivationFunctionType
ALU = mybir.AluOpType

N_CORES = 8
B, C, H, W = 16, 256, 128, 128
N, D, MID = 64, 768, 64
B_PER = B // N_CORES          # 2 batches per core
HW = H * W                    # 16384
P = 128                       # SBUF partitions / channel-block size
NHALF = C // P                # 2 channel halves
KD = D // P                   # 6 contraction chunks for W1
FCH = 4096                    # free-dim chunk of a visual unit
NCH = HW // FCH               # 4 chunks per unit
NEG_BIG = -1.0e30


def _split_multi_waits(nc, max_waits=1):
    # The walrus build here rejects instructions carrying >1 sync-wait
    # ("Too many sync wait commands"); hoist extras onto standalone
    # event-semaphore instructions right before the original.
    n = 0
    for fn in nc.m.functions:
        for blk in fn.blocks:
            out = []
            for inst in blk.instructions:
                si = inst.sync_info
                waits = list(si.on_wait) if si and si.on_wait else []
                if len(waits) > max_waits:
                    for wv in waits[:-max_waits]:
                        n += 1
                        out.append(
                            mybir.InstEventSemaphore(
                                name=f"waitfix-{n}-{inst.name}",
                                engine=inst.engine,
                                ins=[],
                                outs=[],
                                sync_info=mybir.SyncInfo(
                                    on_wait=[wv], on_update=[]
                                ),
                            )
                        )
                    inst.sync_info = mybir.SyncInfo(
                        on_wait=waits[-max_waits:],
                        on_update=list(si.on_update or []),
                    )
                out.append(inst)
            blk.instructions[:] = out
    return n



def _lean_drain_and_barrier(self, tick_clock, wait_clock):
    # Same as TileContext._drain_and_barrier but without the second
    # all-engine barrier: after the single barrier + sem clear, every
    # engine stream simply ends, and NRT won't start a re-execution
    # until all engines (incl. the one doing the clear) have halted.
    from concourse.vector_clock import ScopedClock

    drain_inst = self.nc.sync.drain()
    wait_clock.add_sem_waits(
        drain_inst.ins, ScopedClock({None: tick_clock.global_clock})
    )
    self.nc.all_engine_barrier()
    popped = self.nc._tile_sem_poison_stack.pop()
    assert popped is self._sem_poison
    self.nc.clear_and_free_semaphores(list(self.sems.allocated().values()))


def _build():
    nc = bass.Bass()
    vis = nc.declare_dram_parameter("visual", [B_PER, C, H, W], F32, isOutput=False)
    txt = nc.declare_dram_parameter("text", [B_PER, N, D], F32, isOutput=False)
    msk = nc.declare_dram_parameter("mask", [B_PER, N], I32, isOutput=False)
    w1p = nc.declare_dram_parameter("W1p", [P, KD * MID], F32, isOutput=False)
    b1 = nc.declare_dram_parameter("b1", [MID], F32, isOutput=False)
    w2 = nc.declare_dram_parameter("W2", [MID, C], F32, isOutput=False)
    b2 = nc.declare_dram_parameter("b2", [C], F32, isOutput=False)
    temp = nc.declare_dram_parameter("temperature", [1], F32, isOutput=False)
    out = nc.declare_dram_parameter("out", [B_PER, C, H, W], F32, isOutput=True)

    visf = vis.rearrange("b c h w -> b c (h w)")
    outf = out.rearrange("b c h w -> b c (h w)")

    orig_drain = tile.TileContext._drain_and_barrier
    tile.TileContext._drain_and_barrier = _lean_drain_and_barrier
    try:
        _build_body(nc, visf, outf, txt, msk, w1p, b1, w2, b2, temp)
    finally:
        tile.TileContext._drain_and_barrier = orig_drain
    _split_multi_waits(nc)
    return nc


def _build_body(nc, visf, outf, txt, msk, w1p, b1, w2, b2, temp):
    with tile.TileContext(nc) as tc:
        with (
            tc.tile_pool(name="big", bufs=11) as big,
            tc.tile_pool(name="small", bufs=1) as small,
            tc.tile_pool(name="txtp", bufs=2) as txtp,
            tc.tile_pool(name="punit", bufs=2) as punit,
            tc.tile_pool(name="ptp", bufs=2, space="PSUM") as ptp,
            tc.tile_pool(name="pacc", bufs=1, space="PSUM") as pacc,
            tc.tile_pool(name="pproj", bufs=2, space="PSUM") as pproj,
        ):
            # ---- replicated params ----
            # All small loads go on the scalar HWDGE ring (idle early; the
            # sync ring is reserved for the visual stream) as few, fat
            # descriptors. W1 is pre-packed on host to the [p, (k m)]
            # lhsT-chunk layout so it loads as one contiguous 2D DMA.
            w1s = small.tile([P, KD * MID], F32, tag="w1s")
            nc.gpsimd.dma_start(out=w1s, in_=w1p[:, :])
            # W2 with b2 appended as a 65th contraction row: the proj
            # matmul then adds b2 via a ones row in the rhs.
            w2e = small.tile([MID + 1, C], F32, tag="w2e")
            nc.gpsimd.dma_start(out=w2e[:MID, :], in_=w2[:, :])
            nc.gpsimd.dma_start(
                out=w2e[MID:MID + 1, :], in_=b2.rearrange("(o c) -> o c", o=1)
            )
            # b1 as a single-row lhsT: a 7th accumulation matmul
            # (b1row.T @ ones) adds it to every token column.
            b1r = small.tile([1, MID], F32, tag="b1r")
            nc.gpsimd.dma_start(out=b1r, in_=b1.rearrange("(o m) -> o m", o=1))
            ones1 = small.tile([1, N], F32, tag="ones1")
            nc.vector.memset(ones1, 1.0)
            onesP = small.tile([1, P], F32, tag="onesP")
            nc.vector.memset(onesP, 1.0)
            # rT = 1 / (HW * T) on one partition, then PE-broadcast to 128
            tq = small.tile([1, 1], F32, tag="tq")
            nc.gpsimd.dma_start(out=tq, in_=temp.rearrange("(o t) -> o t", o=1))
            r1 = small.tile([1, 1], F32, tag="r1")
            nc.vector.reciprocal(r1, tq)
            nc.vector.tensor_scalar_mul(r1, r1, 1.0 / float(HW))
            prt = pproj.tile([P, 1], F32, tag="pbc")
            nc.tensor.matmul(prt, lhsT=onesP, rhs=r1, start=True, stop=True)
            rT = small.tile([P, 1], F32, tag="rT")
            nc.scalar.copy(rT, prt)
            ident = small.tile([MID, MID], F32, tag="ident")
            masks.make_identity(nc, ident[:])

            # ---- visual pipeline setup + prologue ----
            # Emitted BEFORE the text path so the early load triggers sit
            # ahead of the text ACT ops in the scalar engine stream (the
            # scalar ring carries the odd-j prologue loads). Two HWDGE
            # rings: a single ring leaves per-transfer completion bubbles
            # (~355 GB/s); two rings sustain ~431 GB/s.
            units = [(b, h) for b in range(B_PER) for h in range(NHALF)]
            NU = len(units)
            vts = {u: [] for u in range(NU)}
            psums = {}
            rings = [nc.sync, nc.scalar]

            def load_chunk(u, j, ring):
                b, h = units[u]
                cs = h * P
                vt = big.tile([P, FCH], F32, tag="vt", name=f"vt{u}_{j}")
                ring.dma_start(
                    out=vt, in_=visf[b, cs:cs + P, j * FCH:(j + 1) * FCH]
                )
                vts[u].append(vt)

            def reduce_chunk(u, j):
                return nc.vector.reduce_sum(
                    psums[u][:, j:j + 1], vts[u][j], axis=AX.X
                )

            def load_reduce(u, j, ring=None):
                load_chunk(u, j, ring or nc.sync)
                return reduce_chunk(u, j)

            # text inputs load first on the scalar ring (tiny; they gate
            # the softmax chain of unit 0 and must not queue behind the
            # 8 MiB of prologue visual loads)
            tfs, mrows = [], []
            for b in range(B_PER):
                tf = txtp.tile([N, D], F32, tag="tf", name=f"tf{b}")
                nc.gpsimd.dma_start(out=tf, in_=txt[b])
                tfs.append(tf)
                mrow = txtp.tile([1, N], I32, tag="mrow", name=f"mrow{b}")
                nc.gpsimd.dma_start(
                    out=mrow, in_=msk[b].rearrange("(o n) -> o n", o=1)
                )
                mrows.append(mrow)

            # prologue: units 0 and 1 load on both rings; unit 0 reduces
            # now, unit 1's reduces are dep-pinned into phase 0
            psums[0] = punit.tile([P, NCH], F32, tag="psums", name="psums0")
            psums[1] = punit.tile([P, NCH], F32, tag="psums", name="psums1")
            for j in range(NCH):
                load_chunk(0, j, rings[j % 2])
            for j in range(NCH):
                load_chunk(1, j, rings[j % 2])
            # unit 0's even reduces on DVE now; odd ones go to ACT after
            # the text path (splitting the serial reduce chain across two
            # engines pulls chain(0) and the first store ~4us earlier)
            for j in (0, 2):
                reduce_chunk(0, j)

            # ---- text path: projT[b][h] = (proj(text[b]) + b2).T ----
            projT = {}
            biasb = []
            for b in range(B_PER):
                tf = tfs[b]
                mrow = mrows[b]
                brow = txtp.tile([1, N], F32, tag="brow")
                nc.gpsimd.tensor_scalar(
                    out=brow, in0=mrow, scalar1=0, scalar2=NEG_BIG,
                    op0=ALU.is_equal, op1=ALU.mult,
                )
                pbb = pproj.tile([P, N], F32, tag="pbc")
                nc.tensor.matmul(pbb, lhsT=onesP, rhs=brow, start=True, stop=True)
                bb = small.tile([P, N], F32, tag=f"biasb{b}")
                nc.scalar.copy(bb, pbb)
                biasb.append(bb)

                tfTs = []
                for k in range(KD):
                    tps = ptp.tile([P, N], F32, tag="tps")
                    nc.tensor.transpose(tps, tf[:, k * P:(k + 1) * P], ident)
                    tfT = txtp.tile([P, N], F32, tag="tfT", bufs=3)
                    nc.scalar.copy(tfT, tps)
                    tfTs.append(tfT)
                hacc = pacc.tile([MID, N], F32, tag="hacc")
                for k in range(KD):
                    nc.tensor.matmul(
                        hacc,
                        lhsT=w1s[:, k * MID:(k + 1) * MID],
                        rhs=tfTs[k],
                        start=(k == 0),
                        stop=False,
                    )
                nc.tensor.matmul(hacc, lhsT=b1r, rhs=ones1, start=False, stop=True)
                # relu(hacc) into rows 0..MID of hTp; row MID is ones so the
                # proj matmul's 65th contraction row adds b2.
                hTp = txtp.tile([MID + 1, N], F32, tag="hTp")
                nc.scalar.activation(hTp[:MID, :], hacc, AF.Relu)
                nc.gpsimd.memset(hTp[MID:MID + 1, :], 1.0)
                for h in range(NHALF):
                    pp = pproj.tile([P, N], F32, tag="pp")
                    nc.tensor.matmul(
                        pp, lhsT=w2e[:, h * P:(h + 1) * P], rhs=hTp,
                        start=True, stop=True,
                    )
                    pt = small.tile([P, N], F32, tag=f"projT{b}{h}")
                    nc.scalar.copy(pt, pp)
                    projT[(b, h)] = pt

            # unit 0 odd reduces on ACT (in-place Copy + accumulator),
            # emitted after the text ACT ops so they don't push projT later
            for j in (1, 3):
                nc.scalar.activation(
                    vts[0][j], vts[0][j], AF.Copy,
                    accum_out=psums[0][:, j:j + 1],
                )

            for u in range(NU):
                b, h = units[u]
                cs = h * P
                # softmax chain -> per-channel gate g (ACT only uses Exp:
                # sigmoid is computed as 1/(1+exp(-x)) to avoid ACT
                # table reloads on the critical path)
                vsum = punit.tile([P, 1], F32, tag="vsum")
                nc.vector.reduce_sum(vsum, psums[u], axis=AX.X)
                vs = punit.tile([P, 1], F32, tag="vs")
                nc.vector.tensor_mul(vs, vsum, rT)
                aff = punit.tile([P, N], F32, tag="aff")
                nc.vector.tensor_scalar_mul(aff, projT[(b, h)], vs)
                nc.vector.tensor_add(aff, aff, biasb[b])
                nmx = punit.tile([P, 1], F32, tag="nmx")
                nc.vector.reduce_max(nmx, aff, axis=AX.X, negate=True)
                ex = punit.tile([P, N], F32, tag="ex")
                sx = punit.tile([P, 1], F32, tag="sx")
                nc.scalar.activation(
                    ex, aff, AF.Exp, bias=nmx, scale=1.0, accum_out=sx
                )
                xp = punit.tile([P, N], F32, tag="xp")
                swx = punit.tile([P, 1], F32, tag="swx")
                nc.vector.tensor_mul(xp, ex, projT[(b, h)])
                nc.vector.reduce_sum(swx, xp, axis=AX.X)
                rs = punit.tile([P, 1], F32, tag="rs")
                nc.vector.reciprocal(rs, sx)
                cw = punit.tile([P, 1], F32, tag="cw")
                nc.vector.tensor_mul(cw, swx, rs)
                en = punit.tile([P, 1], F32, tag="en")
                nc.scalar.activation(en, cw, AF.Exp, scale=-1.0)
                g = punit.tile([P, 1], F32, tag="g")
                nc.vector.tensor_scalar_add(g, en, 1.0)
                nc.vector.reciprocal(g, g)

                if u + 2 < NU:
                    psums[u + 2] = punit.tile(
                        [P, NCH], F32, tag="psums", name=f"psums{u + 2}"
                    )
                for j in range(NCH):
                    s_i = nc.vector.tensor_scalar_mul(vts[u][j], vts[u][j], g)
                    nc.scalar.dma_start(
                        out=outf[b, cs:cs + P, j * FCH:(j + 1) * FCH],
                        in_=vts[u][j],
                    )
                    # units u, u+1 are resident (prologue/previous phase);
                    # this phase streams in u+2 and reduces u+1
                    if u + 2 < NU:
                        load_chunk(u + 2, j, nc.sync)
                    if u + 1 < NU:
                        r_i = reduce_chunk(u + 1, j)
                        # pin DVE order: this unit's scale (and therefore the
                        # softmax chain before it) must precede the next
                        # unit's bulk reduce, or the scheduler slots the
                        # 4.4us reduce into the chain and delays the stores
                        add_dep_helper(r_i.ins, s_i.ins, sync=False,
                                       reason="chain+scale before next reduce")


_NC_CACHE = None


def make_in_maps(inputs):
    visual = np.asarray(inputs["visual_feat"], dtype=np.float32)
    text = np.asarray(inputs["text_feat"], dtype=np.float32)
    mask = np.asarray(inputs["text_mask"], dtype=np.int32)
    w1 = np.asarray(inputs["W1"], dtype=np.float32)
    # pack W1 [768, 64] -> [128, 6*64] so lhsT chunk k for the contraction
    # over D lives at w1p[:, k*64:(k+1)*64] and loads as one contiguous DMA
    w1p = np.ascontiguousarray(
        w1.reshape(KD, P, MID).transpose(1, 0, 2).reshape(P, KD * MID))
    b1 = np.asarray(inputs["b1"], dtype=np.float32)
    w2 = np.asarray(inputs["W2"], dtype=np.float32)
    b2 = np.asarray(inputs["b2"], dtype=np.float32)
    temp = np.asarray(inputs["temperature"], dtype=np.float32)

    in_maps = []
    for i in range(N_CORES):
        s = slice(i * B_PER, (i + 1) * B_PER)
        in_maps.append({
            "visual": np.ascontiguousarray(visual[s]),
            "text": np.ascontiguousarray(text[s]),
            "mask": np.ascontiguousarray(mask[s]),
            "W1p": w1p, "b1": b1, "W2": w2, "b2": b2, "temperature": temp,
        })
    return in_maps


def kernel(**inputs):
    global _NC_CACHE
    if _NC_CACHE is None:
        _NC_CACHE = _build()
    nc = _NC_CACHE
    in_maps = make_in_maps(inputs)
    res = run_bass_kernel_spmd(nc, in_maps, list(range(N_CORES)))
    return np.concatenate([res.results[i]["out"] for i in range(N_CORES)], axis=0)
